# revision 24
# baseline (speedup 1.0000x reference)
"""Channel-attention (bmm-softmax-bmm over channels) on 8 TRN2 NeuronCores.

Math (per batch b):
    q = Wq x + bq 1^T ; k = Wk x + bk 1^T ; v = Wv x + bv 1^T      (x: [C, P])
    E = q k^T ; attn = softmax(E, axis=-1) ; out = attn v

Gram reformulation (cuts MACs ~2.6x):
    G = x x^T (symmetric: only upper-triangle block-row computed)
    s = x @ 1_P
    E = WqT.T @ (G WkT + s bk^T) + bq r^T,   r = Wk s + P bk
    attn_un = exp(E - rowmax), Z = rowsum(attn_un)
    AT = Wv^T attn_un^T ;  t = attn_un @ bv
    out = (AT.T @ x + t 1^T) * (1/Z) per-row

Sharding: data-parallel over B; core i gets batches [2i, 2i+1]; weights
replicated. No cross-core communication.

Implementation notes (measured on HW):
  - Everything fp16, PSUM fp32: fp16's 10-bit mantissa matches the
    softmax-amplified logit path's needs (rel err ~3e-3, gate 2e-2),
    and 2-byte operands stream the PE at 1 col/cycle with FWL weight
    loads; float32r streams at half rate with unhidden 187ns loads.
  - x is fed twice from HBM: natural [C,P] layout for the output matmul
    and host-pretransposed [P,C] for the Gram (no PE transposes of x).
  - Row-sums s = x @ 1 run on the otherwise-idle GpSimd engine as
    partition-axis reduces of the xT chunks, so s never waits for the
    late x quarters and the PE's rank-1 bias matmuls don't stall.
  - One full PSUM bank per concurrent accumulation group: start=True
    clears has_written for the whole bank.
  - Pool-ring slot reuse must be emitted after the previous occupant's
    readers (there is no WAR edge otherwise) — b1's xT loads are
    emitted right after b0's Gram matmuls for the same chunk slot.
"""

import os
from contextlib import ExitStack

import numpy as np

import concourse.bass as bass
from concourse import bacc
import concourse.mybir as mybir
import concourse.tile as tile
from concourse.bass_utils import run_bass_kernel_spmd

B, C, P = 16, 512, 4096
N_CORES = 8
BPC = B // N_CORES           # batches per core
CT = C // 128                # 4 c-tiles
QTR = 1024                   # x quarter width
NQ = P // QTR                # 4 quarters per batch
CHT = 8                      # p-tiles per xT chunk
NCH = P // (128 * CHT)       # 4 chunks
F32 = mybir.dt.float32
FP16 = mybir.dt.float16

AX = mybir.AxisListType
ALU = mybir.AluOpType
ACTF = mybir.ActivationFunctionType


def build_nc():
    nc = bacc.Bacc(trn_type="TRN2", target_bir_lowering=False, debug=False)

    x_d = nc.dram_tensor("x", [BPC, C, P], FP16, kind="ExternalInput")
    xt_d = nc.dram_tensor("xt", [BPC, P, C], FP16, kind="ExternalInput")
    wqt_d = nc.dram_tensor("wqt", [C, C], FP16, kind="ExternalInput")
    wkt_d = nc.dram_tensor("wkt", [C, C], FP16, kind="ExternalInput")
    wv_d = nc.dram_tensor("wv", [C, C], FP16, kind="ExternalInput")
    bqr_d = nc.dram_tensor("bq_row", [1, C], FP16, kind="ExternalInput")
    bkr_d = nc.dram_tensor("bk_row", [1, C], FP16, kind="ExternalInput")
    pbk_d = nc.dram_tensor("pbk_row", [1, C], FP16, kind="ExternalInput")
    bvr_d = nc.dram_tensor("bv_row", [1, C], FP16, kind="ExternalInput")
    identh_d = nc.dram_tensor("identh", [128, 128], FP16, kind="ExternalInput")
    out_d = nc.dram_tensor("out", [BPC, C, P], FP16, kind="ExternalOutput")

    DBG = bool(os.environ.get("KDBG"))
    if DBG:
        dbg_g = nc.dram_tensor("dbg_g", [BPC, 128, CT, C], FP16, kind="ExternalOutput")
        dbg_u = nc.dram_tensor("dbg_u", [BPC, 128, CT, C], FP16, kind="ExternalOutput")
        dbg_a = nc.dram_tensor("dbg_a", [BPC, 128, CT, C], FP16, kind="ExternalOutput")
        dbg_at = nc.dram_tensor("dbg_at", [BPC, 128, CT, C], FP16, kind="ExternalOutput")
        dbg_s = nc.dram_tensor("dbg_s", [BPC, 1, C], FP16, kind="ExternalOutput")
        dbg_r = nc.dram_tensor("dbg_r", [BPC, 1, C], FP16, kind="ExternalOutput")

    with ExitStack() as ctx:
        tc = ctx.enter_context(tile.TileContext(nc))
        const = ctx.enter_context(tc.tile_pool(name="const", bufs=1))
        xpool = ctx.enter_context(tc.tile_pool(name="xpool", bufs=6))
        xtp = ctx.enter_context(tc.tile_pool(name="xtp", bufs=4))
        gsbp = ctx.enter_context(tc.tile_pool(name="gsbp", bufs=2))
        usbp = ctx.enter_context(tc.tile_pool(name="usbp", bufs=2))
        atnp = ctx.enter_context(tc.tile_pool(name="atnp", bufs=2))
        atntp = ctx.enter_context(tc.tile_pool(name="atntp", bufs=2))
        atp = ctx.enter_context(tc.tile_pool(name="atp", bufs=2))
        vecp = ctx.enter_context(tc.tile_pool(name="vecp", bufs=2))
        sgp = ctx.enter_context(tc.tile_pool(name="sgp", bufs=4))
        outp = ctx.enter_context(tc.tile_pool(name="outp", bufs=4))
        gps = ctx.enter_context(tc.tile_pool(name="gps", bufs=1, space="PSUM"))
        ops = ctx.enter_context(tc.tile_pool(name="ops", bufs=2, space="PSUM"))
        mmps = ctx.enter_context(tc.tile_pool(name="mmps", bufs=2, space="PSUM"))

        st0, st1 = {}, {}

        # ---- DMA loads (sync queue, ordered by first need) ----
        def load_xt(b, ch, st, split=1):
            xtt = xtp.tile([128, CHT, C], FP16, name=f"xt_b{b}c{ch}", tag="xt")
            w = CHT // split
            for s in range(split):
                r0 = ch * CHT * 128 + s * w * 128
                nc.sync.dma_start(
                    out=xtt[:, s * w : (s + 1) * w, :],
                    in_=xt_d[b, r0 : r0 + w * 128, :].rearrange(
                        "(n p) c -> p n c", p=128
                    ),
                )
            st[f"xt{ch}"] = xtt

        def load_x(b, q, st):
            xt_ = xpool.tile([128, CT, QTR], FP16, name=f"x_b{b}q{q}", tag="x")
            nc.sync.dma_start(
                out=xt_,
                in_=x_d[b, :, q * QTR : (q + 1) * QTR].rearrange(
                    "(t p) f -> p t f", p=128
                ),
            )
            st[f"x{q}"] = xt_

        load_xt(0, 0, st0, split=2)
        load_xt(0, 1, st0)
        load_xt(0, 2, st0)
        load_xt(0, 3, st0)

        identh = const.tile([128, 128], FP16, name="identh")
        nc.sync.dma_start(out=identh, in_=identh_d[:, :])
        ones11_f = const.tile([1, 1], F32, name="ones11_f")
        nc.vector.memset(ones11_f, 1.0)
        ones11 = const.tile([1, 1], FP16, name="ones11")
        nc.vector.tensor_copy(ones11, ones11_f)

        def load_w(name, d):
            t = const.tile([128, CT, C], FP16, name=name)
            nc.sync.dma_start(out=t, in_=d[:, :].rearrange("(t p) f -> p t f", p=128))
            return t

        wkt_sb = load_w("wkt_sb", wkt_d)
        bkr_sb = const.tile([1, C], FP16, name="bkr_sb")
        nc.sync.dma_start(out=bkr_sb, in_=bkr_d[:, :])
        pbk_sb = const.tile([1, C], FP16, name="pbk_sb")
        nc.sync.dma_start(out=pbk_sb, in_=pbk_d[:, :])
        wqt_sb = load_w("wqt_sb", wqt_d)
        bqr_sb = const.tile([1, C], FP16, name="bqr_sb")
        nc.sync.dma_start(out=bqr_sb, in_=bqr_d[:, :])
        wv_sb = load_w("wv_sb", wv_d)
        bv_rep = const.tile([128, C], FP16, name="bv_rep")
        nc.sync.dma_start(out=bv_rep, in_=bvr_d[:, :].partition_broadcast(128))

        # ---- per-batch phases ----
        def G_mms(b, ch, st):
            """Gram accumulation for one xT chunk (8 p-tiles), upper
            triangle block-rows; one full PSUM bank per row block."""
            if ch == 0:
                st["G_ps"] = [
                    gps.tile([128, 512], F32, name=f"G{cc}_b{b}", tag=f"G{cc}")
                    for cc in range(CT)
                ]
            outs = [st["G_ps"][cc][:, : 512 - cc * 128] for cc in range(CT)]
            xtt = st[f"xt{ch}"]
            for n in range(CHT):
                first = ch == 0 and n == 0
                last = ch == NCH - 1 and n == CHT - 1
                for cc in range(CT):
                    nc.tensor.matmul(
                        out=outs[cc],
                        lhsT=xtt[:, n, cc * 128 : (cc + 1) * 128],
                        rhs=xtt[:, n, cc * 128 :],
                        start=first,
                        stop=last,
                    )

        def sred_gp(b, ch, st):
            """s += per-chunk column sums of xT, on GpSimd (partition-axis
            reduce): decoupled from the late x quarters."""
            xtt = st[f"xt{ch}"]
            ps = [sgp.tile([1, C], F32, name=f"rp{i}", tag="redp") for i in range(4)]
            for i in range(4):
                ra = sgp.tile([1, C], F32, name="ra", tag="red")
                rb = sgp.tile([1, C], F32, name="rb", tag="red")
                nc.gpsimd.reduce_sum(out=ra, in_=xtt[:, 2 * i, :], axis=AX.C)
                nc.gpsimd.reduce_sum(out=rb, in_=xtt[:, 2 * i + 1, :], axis=AX.C)
                nc.gpsimd.tensor_add(ps[i], ra, rb)
            q0 = sgp.tile([1, C], F32, name="rq0", tag="redq", bufs=2)
            q1 = sgp.tile([1, C], F32, name="rq1", tag="redq", bufs=2)
            nc.gpsimd.tensor_add(q0, ps[0], ps[1])
            nc.gpsimd.tensor_add(q1, ps[2], ps[3])
            cs = sgp.tile([1, C], F32, name=f"cs_b{b}c{ch}", tag=f"cs{ch}", bufs=2)
            nc.gpsimd.tensor_add(cs, q0, q1)
            st[f"cs{ch}"] = cs

        def s_finish(b, st):
            sa = sgp.tile([1, C], F32, name="sfa", tag="sfa", bufs=2)
            sb_ = sgp.tile([1, C], F32, name="sfb", tag="sfb", bufs=2)
            nc.gpsimd.tensor_add(sa, st["cs0"], st["cs1"])
            nc.gpsimd.tensor_add(sb_, st["cs2"], st["cs3"])
            srow = vecp.tile([1, C], FP16, name="srow", tag="srow", bufs=2)
            nc.gpsimd.tensor_add(srow, sa, sb_)
            st["srow"] = srow
            if DBG:
                nc.sync.dma_start(out=dbg_s[b], in_=srow)

        def scol_T(b, st):
            # scol (s as a [128, CT] column) from srow: rank-1 matmuls
            # against ones transpose each 128-wide row slice into a column.
            scol_ps = mmps.tile([128, CT], F32, name="scol_ps", tag="mm")
            for t in range(CT):
                nc.tensor.matmul(
                    out=scol_ps[:, t : t + 1],
                    lhsT=st["srow"][:, t * 128 : (t + 1) * 128],
                    rhs=ones11,
                    start=True,
                    stop=True,
                )
            scol = vecp.tile([128, CT], FP16, name="scol", tag="scol")
            nc.vector.tensor_copy(scol, scol_ps)
            st["scol"] = scol

        def G_evac(b, st):
            G_sb = gsbp.tile([128, CT, C], FP16, name="G_sb", tag="gsb")
            for cc in range(CT):
                nc.vector.tensor_copy(
                    G_sb[:, cc, cc * 128 : 512], st["G_ps"][cc][:, : 512 - cc * 128]
                )
            st["G_sb"] = G_sb
            del st["G_ps"]

        def G_mirror(b, st):
            """Mirror strictly-lower blocks via PE transposes (G symmetric)."""
            G_sb = st["G_sb"]
            pairs = [(dd, cc) for cc in range(CT) for dd in range(cc)]
            lps = [
                mmps.tile([128, 384], FP16, name=f"lps{i}", tag="mm")
                for i in range(2)
            ]
            for i, (dd, cc) in enumerate(pairs):
                nc.tensor.transpose(
                    out=lps[i // 3][:, (i % 3) * 128 : (i % 3 + 1) * 128],
                    in_=G_sb[:, dd, cc * 128 : (cc + 1) * 128],
                    identity=identh,
                )
            for i, (dd, cc) in enumerate(pairs):
                nc.vector.tensor_copy(
                    G_sb[:, cc, dd * 128 : (dd + 1) * 128],
                    lps[i // 3][:, (i % 3) * 128 : (i % 3 + 1) * 128],
                )
            if DBG:
                nc.sync.dma_start(out=dbg_g[b], in_=G_sb)

        def U_phase(b, st):
            U_sb = usbp.tile([128, CT, C], FP16, name="U_sb", tag="usb")
            for ic in range(CT):
                u_ps = ops.tile([128, C], F32, name="u_ps", tag="out")
                for e in range(CT):
                    nc.tensor.matmul(
                        out=u_ps,
                        lhsT=st["G_sb"][:, e, ic * 128 : (ic + 1) * 128],
                        rhs=wkt_sb[:, e, :],
                        start=(e == 0),
                        stop=False,
                    )
                nc.tensor.matmul(
                    out=u_ps,
                    lhsT=st["srow"][:, ic * 128 : (ic + 1) * 128],
                    rhs=bkr_sb,
                    start=False,
                    stop=True,
                )
                nc.scalar.copy(U_sb[:, ic, :], u_ps)
            st["U_sb"] = U_sb
            # r = Wk s + P bk   (as a row [1, C])
            scol_T(b, st)
            r_ps = mmps.tile([1, C], F32, name="r_ps", tag="mm")
            for t in range(CT):
                nc.tensor.matmul(
                    out=r_ps,
                    lhsT=st["scol"][:, t : t + 1],
                    rhs=wkt_sb[:, t, :],
                    start=(t == 0),
                    stop=False,
                )
            nc.tensor.matmul(
                out=r_ps, lhsT=ones11, rhs=pbk_sb, start=False, stop=True
            )
            rrow = vecp.tile([1, C], FP16, name="rrow", tag="rrow", bufs=2)
            nc.vector.tensor_copy(rrow, r_ps)
            st["rrow"] = rrow
            if DBG:
                nc.sync.dma_start(out=dbg_u[b], in_=U_sb)
                nc.sync.dma_start(out=dbg_r[b], in_=rrow)

        def E_softmax(b, st):
            """E matmuls; softmax reads the PSUM bank directly."""
            attn_sb = atnp.tile([128, CT, C], FP16, name="attn_sb", tag="atn")
            mx = vecp.tile([128, CT], F32, name="mx", tag="mx")
            negm = vecp.tile([128, CT], F32, name="negm", tag="negm")
            zsum = vecp.tile([128, CT], F32, name="zsum", tag="zsum")
            recip = vecp.tile([128, CT], F32, name="recip", tag="recip")
            for cc in range(CT):
                e_ps = ops.tile([128, C], F32, name="e_ps", tag="out")
                for i in range(CT):
                    nc.tensor.matmul(
                        out=e_ps,
                        lhsT=wqt_sb[:, i, cc * 128 : (cc + 1) * 128],
                        rhs=st["U_sb"][:, i, :],
                        start=(i == 0),
                        stop=False,
                    )
                nc.tensor.matmul(
                    out=e_ps,
                    lhsT=bqr_sb[:, cc * 128 : (cc + 1) * 128],
                    rhs=st["rrow"],
                    start=False,
                    stop=True,
                )
                nc.vector.reduce_max(out=mx[:, cc : cc + 1], in_=e_ps, axis=AX.X)
                nc.vector.tensor_scalar_mul(
                    negm[:, cc : cc + 1], mx[:, cc : cc + 1], -1.0
                )
                nc.scalar.activation(
                    out=attn_sb[:, cc, :],
                    in_=e_ps,
                    func=ACTF.Exp,
                    bias=negm[:, cc : cc + 1],
                    scale=1.0,
                    accum_out=zsum[:, cc : cc + 1],
                )
            nc.vector.reciprocal(out=recip, in_=zsum)
            st["attn"] = attn_sb
            st["recip"] = recip
            if DBG:
                nc.sync.dma_start(out=dbg_a[b], in_=attn_sb)

        def attnT_AT(b, st):
            attnT_sb = atntp.tile([128, CT, C], FP16, name="attnT_sb", tag="atnt")
            for dc in range(CT):
                at_ps = mmps.tile([128, C], FP16, name="at_ps", tag="mm")
                for t in range(CT):
                    nc.tensor.transpose(
                        out=at_ps[:, t * 128 : (t + 1) * 128],
                        in_=st["attn"][:, t, dc * 128 : (dc + 1) * 128],
                        identity=identh,
                    )
                nc.vector.tensor_copy(attnT_sb[:, dc, :], at_ps)
            AT_sb = atp.tile([128, CT, C], FP16, name="AT_sb", tag="at")
            for ic in range(CT):
                a_ps = ops.tile([128, C], F32, name="a_ps", tag="out")
                for d in range(CT):
                    nc.tensor.matmul(
                        out=a_ps,
                        lhsT=wv_sb[:, d, ic * 128 : (ic + 1) * 128],
                        rhs=attnT_sb[:, d, :],
                        start=(d == 0),
                        stop=(d == CT - 1),
                    )
                nc.scalar.copy(AT_sb[:, ic, :], a_ps)
            st["AT"] = AT_sb
            # t = attn_un @ bv as per-partition dot products on DVE
            tts = vecp.tile([128, C], F32, name="tts", tag="tts", bufs=1)
            tcol = vecp.tile([128, CT], F32, name="tcol", tag="tcol")
            for cc in range(CT):
                nc.vector.tensor_mul(tts, st["attn"][:, cc, :], bv_rep)
                nc.vector.reduce_sum(out=tcol[:, cc : cc + 1], in_=tts, axis=AX.X)
            rt = vecp.tile([128, CT], F32, name="rt", tag="rt")
            nc.vector.tensor_mul(rt, tcol, st["recip"])
            st["rt"] = rt
            if DBG:
                nc.sync.dma_start(out=dbg_at[b], in_=AT_sb)

        def out_phase(b, q, st, fine=False):
            for cc in range(CT):
                stage = outp.tile([128, QTR], FP16, name="stage", tag="stage")
                for pb in range(2):
                    o_ps = ops.tile([128, 512], F32, name="o_ps", tag="out")
                    for i in range(CT):
                        nc.tensor.matmul(
                            out=o_ps,
                            lhsT=st["AT"][:, i, cc * 128 : (cc + 1) * 128],
                            rhs=st[f"x{q}"][:, i, pb * 512 : (pb + 1) * 512],
                            start=(i == 0),
                            stop=(i == CT - 1),
                        )
                    if pb % 2 == 0:
                        nc.scalar.activation(
                            out=stage[:, pb * 512 : (pb + 1) * 512],
                            in_=o_ps,
                            func=ACTF.Identity,
                            bias=st["rt"][:, cc : cc + 1],
                            scale=st["recip"][:, cc : cc + 1],
                        )
                    else:
                        nc.vector.tensor_scalar(
                            out=stage[:, pb * 512 : (pb + 1) * 512],
                            in0=o_ps,
                            scalar1=st["recip"][:, cc : cc + 1],
                            scalar2=st["rt"][:, cc : cc + 1],
                            op0=ALU.mult,
                            op1=ALU.add,
                        )
                    if fine:
                        nc.sync.dma_start(
                            out=out_d[
                                b,
                                cc * 128 : (cc + 1) * 128,
                                q * QTR + pb * 512 : q * QTR + (pb + 1) * 512,
                            ],
                            in_=stage[:, pb * 512 : (pb + 1) * 512],
                        )
                if not fine:
                    nc.sync.dma_start(
                        out=out_d[
                            b, cc * 128 : (cc + 1) * 128, q * QTR : (q + 1) * QTR
                        ],
                        in_=stage,
                    )

        # ---- schedule ----
        for ch in range(NCH):
            G_mms(0, ch, st0)
            sred_gp(0, ch, st0)
            # b1's chunk reuses this chunk's SBUF slot: must be emitted
            # after its readers (G matmuls + gpsimd reduce).
            load_xt(1, ch, st1)
            if ch > 0:
                load_x(0, ch - 1, st0)
        load_x(0, 3, st0)
        s_finish(0, st0)
        G_evac(0, st0)
        G_mirror(0, st0)
        U_phase(0, st0)
        E_softmax(0, st0)
        G_mms(1, 0, st1)           # covers b0 softmax latency
        sred_gp(1, 0, st1)
        attnT_AT(0, st0)
        G_mms(1, 1, st1)
        sred_gp(1, 1, st1)
        out_phase(0, 0, st0)
        load_x(1, 0, st1)
        G_mms(1, 2, st1)
        sred_gp(1, 2, st1)
        out_phase(0, 1, st0)
        load_x(1, 1, st1)
        G_mms(1, 3, st1)
        sred_gp(1, 3, st1)
        out_phase(0, 2, st0)
        load_x(1, 2, st1)
        s_finish(1, st1)
        G_evac(1, st1)
        G_mirror(1, st1)
        U_phase(1, st1)
        E_softmax(1, st1)
        out_phase(0, 3, st0)       # covers b1 softmax latency
        load_x(1, 3, st1)
        attnT_AT(1, st1)
        out_phase(1, 0, st1)
        out_phase(1, 1, st1)
        out_phase(1, 2, st1)
        out_phase(1, 3, st1, fine=True)

    nc.compile()
    return nc


_CACHE = {}


def _get_nc():
    if "nc" not in _CACHE:
        _CACHE["nc"] = build_nc()
    return _CACHE["nc"]


def make_in_maps(x, Wq, bq, Wk, bk, Wv, bv):
    x = np.asarray(x, np.float32)
    x_h = x.astype(np.float16)
    xt_h = np.ascontiguousarray(x_h.transpose(0, 2, 1))
    Wq = np.asarray(Wq, np.float32)
    Wk = np.asarray(Wk, np.float32)
    bq = np.asarray(bq, np.float32)
    bk = np.asarray(bk, np.float32)
    shared = {
        "wqt": np.ascontiguousarray(Wq.T.astype(np.float16)),
        "wkt": np.ascontiguousarray(Wk.T.astype(np.float16)),
        "wv": np.ascontiguousarray(np.asarray(Wv, np.float16)),
        "bq_row": np.ascontiguousarray(bq[None, :].astype(np.float16)),
        "bk_row": np.ascontiguousarray(bk[None, :].astype(np.float16)),
        "pbk_row": np.ascontiguousarray((float(P) * bk)[None, :].astype(np.float16)),
        "bv_row": np.ascontiguousarray(np.asarray(bv, np.float16)[None, :]),
        "identh": np.eye(128, dtype=np.float16),
    }
    return [
        {
            "x": np.ascontiguousarray(x_h[BPC * i : BPC * (i + 1)]),
            "xt": np.ascontiguousarray(xt_h[BPC * i : BPC * (i + 1)]),
            **shared,
        }
        for i in range(N_CORES)
    ]


def run(inputs, trace=False, tmpdir=None):
    nc = _get_nc()
    in_maps = make_in_maps(**inputs)
    res = run_bass_kernel_spmd(
        nc, in_maps, core_ids=list(range(N_CORES)), trace=trace, tmpdir=tmpdir
    )
    out = np.concatenate([res.results[i]["out"] for i in range(N_CORES)], axis=0)
    return out.astype(np.float32), res


def kernel(**inputs) -> np.ndarray:
    out, _ = run(inputs, trace=False)
    return out
